# revision 4
# baseline (speedup 1.0000x reference)
"""Trainium2 Bass kernel for DynamicCondLinear (MoE-routing style).

Math: condition batch is 1, so the softmax routing weights (K=8) are shared by
all 32 samples; out = sum_k a_k * (x @ W_k^T + kb_k) with
a = softmax(relu(cond @ w1 + b1) @ w2 + b2).

v2 design (vs the 44 us fp16 baseline):
 - Weight stream in fp8 E3M4 (4-bit mantissa): half the HBM bytes of fp16,
   measured end-to-end rel err 1.15e-2 (gate is 2e-2). W is pre-scaled by
   2^8 on the host (exact pow2, folded back via the mask constant).
 - UNSCALED xT stationaries: main matmuls no longer depend on the softmax.
   Per-expert psum regions (4 col-tiles x 32 batch rows each); the alphas
   are applied at PSUM-evac time (fused scale-add on DVE), so the whole
   alpha chain is off the critical path.
 - Col-tiled main matmuls: out rows live at psum partitions 32t..32t+31 for
   tile t = j%4, so 4 independent 128x32 PE tiles stream concurrently
   (PE busy ~3.5 us < DMA ~12.5 us). A final mask-matmul folds the 4
   partition groups back to (32, 256).
 - The alpha MLP's w1 is SHARDED over H across the 8 cores (64 cols each,
   fp8 E3M4); partial scores (1,8) are AllReduce'd via collective_compute.
   This removes the 2 MB/core replicated w1 load from the stream.
 - Bias enters the main accumulation as an extra contract row (xe row-0
   ones stationary vs a kb slab), no transposes needed.

Host-side prep is layout/cast only (transpose/reshape/pow2-scale/cast).
"""

import os
import sys

import numpy as np

if "/opt/trn_rl_repo" not in sys.path:
    sys.path.insert(0, "/opt/trn_rl_repo")

import concourse.bacc as bacc
import concourse.mybir as mybir
import concourse.tile as tile
from concourse.bass_utils import run_bass_kernel_spmd

B, IN, OUT, K, H = 32, 2048, 2048, 8, 512
NCORES = 8
OC = OUT // NCORES   # 256 out channels per core
JT = IN // 128       # 16 contraction tiles
HS = H // NCORES     # 64 hidden units per core (MLP shard)

F32 = mybir.dt.float32
BF16 = mybir.dt.bfloat16
FP16 = mybir.dt.float16
FP8 = mybir.dt.float8e3  # E3M4

WSCALE = 2.0 ** 8    # W * 256 -> e3m4 sweet spot; folded into mask constant
W1SCALE = 2.0 ** 5   # w1 * 32; folded into w2 host-side

# knobs
COLTILE = os.environ.get("KERNEL_COLTILE", "1") == "1"
SHARD_MLP = os.environ.get("KERNEL_SHARD_MLP", "1") == "1"
NWARM = int(os.environ.get("KERNEL_WARMUP", "16"))

_CACHE = {}
LAST_RESULTS = None  # test.py reads this for profiling info


def _build_module():
    nc = bacc.Bacc("TRN2", target_bir_lowering=False, debug=False,
                   num_devices=NCORES)

    wt_d = nc.dram_tensor("wt", (K, 128, JT * OC), FP8, kind="ExternalInput")
    kbs_d = nc.dram_tensor("kbs", (1, K * OC), FP8, kind="ExternalInput")
    xt_d = nc.dram_tensor("xt", (128, JT * B), FP16, kind="ExternalInput")
    ct_d = nc.dram_tensor("ct", (128, JT), FP16, kind="ExternalInput")
    w1st_d = nc.dram_tensor("w1st", (128, JT * HS), FP8, kind="ExternalInput")
    w2st_d = nc.dram_tensor("w2st", (128, K), BF16, kind="ExternalInput")
    b1c_d = nc.dram_tensor("b1c", (HS, 1), F32, kind="ExternalInput")
    b2r_d = nc.dram_tensor("b2r", (1, K), F32, kind="ExternalInput")
    mask_d = nc.dram_tensor("mask", (128, B), FP16, kind="ExternalInput")
    y_d = nc.dram_tensor("y", (B, OC), F32, kind="ExternalOutput")
    # warmup sink: consumed so bacc's DCE keeps the PE warm-up matmuls
    ysink_d = nc.dram_tensor("ysink", (1, 1), F32, kind="ExternalOutput")

    # collective staging (internal DRAM; output must be Shared)
    cc_in = nc.dram_tensor("cc_in", (1, K), F32)
    cc_out = nc.dram_tensor("cc_out", (1, K), F32, addr_space="Shared")

    with tile.TileContext(nc) as tc:
        with (
            tc.tile_pool(name="cpool", bufs=1) as cpool,
            tc.tile_pool(name="wpool", bufs=1) as wpool,
            tc.tile_pool(name="ppool", bufs=1, space="PSUM") as ppool,
        ):
            # ---------- DMA: small loads on the scalar (ACT) ring ----------
            xt_sb = cpool.tile((128, JT * B), FP16)
            nc.scalar.dma_start(xt_sb[:], xt_d.ap())
            ct_sb = cpool.tile((128, JT), FP16)
            nc.scalar.dma_start(ct_sb[:], ct_d.ap())
            w1st_sb = cpool.tile((128, JT * HS), FP8)
            nc.scalar.dma_start(w1st_sb[:], w1st_d.ap())
            w2st_sb = cpool.tile((128, K), BF16)
            nc.scalar.dma_start(w2st_sb[:], w2st_d.ap())
            b1c_sb = cpool.tile((HS, 1), F32)
            nc.scalar.dma_start(b1c_sb[:], b1c_d.ap())
            b2r_sb = cpool.tile((1, K), F32)
            nc.scalar.dma_start(b2r_sb[:], b2r_d.ap())
            mask_sb = cpool.tile((128, B), FP16)
            nc.scalar.dma_start(mask_sb[:], mask_d.ap())

            # ---------- DMA: weight stream on the sync ring ----------
            slabs = []
            for k in range(K):
                slab = wpool.tile((128, JT * OC), FP8, tag="wt_slab", bufs=K)
                if k == 0:  # fine-grained head chunks so the PE starts early
                    q = JT * OC // 4
                    for c in range(4):
                        nc.sync.dma_start(slab[:, c * q:(c + 1) * q],
                                          wt_d.ap()[k][:, c * q:(c + 1) * q])
                else:
                    hj = JT * OC // 2
                    nc.sync.dma_start(slab[:, :hj], wt_d.ap()[k][:, :hj])
                    nc.sync.dma_start(slab[:, hj:], wt_d.ap()[k][:, hj:])
                slabs.append(slab)
                if k == 0:
                    kbs_sb = wpool.tile((128, K * OC), FP8, tag="kbs")
                    nc.gpsimd.memset(kbs_sb[:], 0.0)
                    nc.sync.dma_start(kbs_sb[0:1, :], kbs_d.ap())

            # xe stationary: row 0 = ones (bias contract row)
            xe_sb = cpool.tile((128, B), FP16)
            nc.gpsimd.memset(xe_sb[:], 0.0)
            nc.gpsimd.memset(xe_sb[0:1, :], 1.0)

            # ---------- PE warm-up (HAM): dep-free matmuls ----------
            dum_a = cpool.tile((128, B), FP16)
            nc.gpsimd.memset(dum_a[:], 0.0)
            dum_b = cpool.tile((128, OC), FP8)
            nc.gpsimd.memset(dum_b[:], 0.0)
            dum_psum = ppool.tile((B, OC), F32, tag="pdum")
            dum_sink = cpool.tile((1, 1), F32)
            for _ in range(NWARM):
                nc.tensor.matmul(dum_psum[:], dum_a[:], dum_b[:],
                                 start=True, stop=True)

            # ---------- alpha MLP (sharded over H) ----------
            # h_col (HS,1) = sum_j w1st[:, j].T @ ct[:, j]  (two 32-col groups)
            psum_hc = ppool.tile((128, 1), F32, tag="pA")
            for half in range(2):
                sl = slice(half * 32, half * 32 + 32)
                for j in range(JT):
                    nc.tensor.matmul(
                        psum_hc[sl, :],
                        w1st_sb[:, j * HS + half * 32:j * HS + half * 32 + 32],
                        ct_sb[:, j:j + 1],
                        start=(j == 0), stop=(j == JT - 1),
                    )
            # hz (128,1): relu(h + b1) in rows 0..HS-1, zeros elsewhere
            hz_sb = cpool.tile((128, 1), FP16)
            nc.gpsimd.memset(hz_sb[:], 0.0)
            nc.scalar.activation(hz_sb[0:HS, :], psum_hc[0:HS, :],
                                 mybir.ActivationFunctionType.Relu,
                                 bias=b1c_sb[:])
            # partial scores (1, K)
            psum_s = ppool.tile((1, K), F32, tag="pB")
            nc.tensor.matmul(psum_s[:], hz_sb[:], w2st_sb[:],
                             start=True, stop=True)
            s_sb = cpool.tile((1, K), F32)
            nc.vector.tensor_copy(s_sb[:], psum_s[:])

            # ---------- score AllReduce across the 8 cores ----------
            if SHARD_MLP:
                nc.scalar.dma_start(cc_in.ap(), s_sb[:])
                nc.gpsimd.collective_compute(
                    "AllReduce",
                    mybir.AluOpType.add,
                    ins=[cc_in.ap()],
                    outs=[cc_out.ap()],
                    replica_groups=[list(range(NCORES))],
                )
                sfull_sb = cpool.tile((1, K), F32)
                nc.scalar.dma_start(sfull_sb[:], cc_out.ap())
            else:
                sfull_sb = s_sb

            # broadcast scores to 128 partitions, softmax numerator
            s_b = cpool.tile((128, K), F32)
            nc.gpsimd.partition_broadcast(s_b[:], sfull_sb[:])
            b2b_sb = cpool.tile((128, K), F32)
            nc.gpsimd.partition_broadcast(b2b_sb[:], b2r_sb[:])
            s_b2 = cpool.tile((128, K), F32)
            nc.vector.tensor_add(s_b2[:], s_b[:], b2b_sb[:])
            e_b = cpool.tile((128, K), F32)
            nc.scalar.activation(e_b[:], s_b2[:],
                                 mybir.ActivationFunctionType.Exp)
            esum = cpool.tile((128, 1), F32)
            nc.vector.reduce_sum(esum[:], e_b[:], axis=mybir.AxisListType.X)
            rinv = cpool.tile((128, 1), F32)
            nc.vector.reciprocal(rinv[:], esum[:])

            # ---------- main stream: per-expert psum regions ----------
            # YK[32t+b, k*OC+o] accumulates sum_{j: j%4==t} xT_j.T @ W_k[j]
            PR = 128 if COLTILE else 32  # active psum partition rows
            yk_psum = ppool.tile((128, K * OC), F32, tag="pyk")
            for k in range(K):
                kc = slice(k * OC, (k + 1) * OC)
                for j in range(JT):
                    t = j % 4 if COLTILE else 0
                    rows = slice(32 * t, 32 * t + 32)
                    if COLTILE:
                        first = j < 4
                        # tile 0's group is closed by the bias mm below
                        last = (j >= JT - 4) and t != 0
                    else:
                        first = j == 0
                        last = False
                    nc.tensor.matmul(
                        yk_psum[rows, kc],
                        xt_sb[:, j * B:(j + 1) * B],
                        slabs[k][:, j * OC:(j + 1) * OC],
                        start=first, stop=last,
                        tile_position=(0, 32 * t),
                    )
                # bias row (once, into tile 0's rows; closes tile0's group)
                nc.tensor.matmul(yk_psum[0:32, kc], xe_sb[:], kbs_sb[:, kc],
                                 start=False, stop=True)

                # fused alpha scale-accumulate off psum (hidden under stream)
                if k == 0:
                    s_acc = cpool.tile((128, OC), FP16, tag="sacc", bufs=2)
                    nc.vector.tensor_scalar_mul(s_acc[0:PR, :],
                                                yk_psum[0:PR, kc],
                                                e_b[0:PR, 0:1])
                    prev = s_acc
                else:
                    nxt = cpool.tile((128, OC), FP16, tag="sacc", bufs=2)
                    nc.vector.scalar_tensor_tensor(
                        nxt[0:PR, :], yk_psum[0:PR, kc], e_b[0:PR, k:k + 1],
                        prev[0:PR, :],
                        op0=mybir.AluOpType.mult, op1=mybir.AluOpType.add)
                    prev = nxt

            y_sb = cpool.tile((B, OC), F32)
            if COLTILE:
                # fold the 4 col-tile partition groups: y = mask.T @ S
                # (mask carries the 1/WSCALE factor)
                y_psum = ppool.tile((B, OC), F32, tag="pB")
                nc.tensor.matmul(y_psum[:], mask_sb[:], prev[:],
                                 start=True, stop=True)
                nc.scalar.activation(y_sb[:], y_psum[:],
                                     mybir.ActivationFunctionType.Copy,
                                     scale=rinv[0:B, :])
            else:
                rinv2 = cpool.tile((128, 1), F32)
                nc.vector.tensor_scalar_mul(rinv2[:], rinv[:], 1.0 / WSCALE)
                nc.scalar.activation(y_sb[:], prev[0:B, :],
                                     mybir.ActivationFunctionType.Copy,
                                     scale=rinv2[0:B, :])
            nc.scalar.dma_start(y_d.ap(), y_sb[:])
            nc.vector.tensor_copy(dum_sink[:], dum_psum[0:1, 0:1])
            nc.scalar.dma_start(ysink_d.ap(), dum_sink[:])

    nc.compile()
    return nc


def _prep_inputs(x, condition, w1, b1, w2, b2, kernels_weights, kernels_bias):
    """Layout-only host prep: per-core shards, retile/cast for DMA."""
    import ml_dtypes
    bf16 = ml_dtypes.bfloat16
    fp8 = ml_dtypes.float8_e3m4
    f = np.float32
    x = np.asarray(x, f)
    condition = np.asarray(condition, f)
    w1 = np.asarray(w1, f)
    b1 = np.asarray(b1, f)
    w2 = np.asarray(w2, f)
    b2 = np.asarray(b2, f)
    kernels_weights = np.asarray(kernels_weights, f)
    kernels_bias = np.asarray(kernels_bias, f)

    # xT tiled: xt[p, j*B + b] = x[b, j*128 + p]
    xt = np.ascontiguousarray(
        x.T.reshape(JT, 128, B).transpose(1, 0, 2)).reshape(128, JT * B)
    xt = xt.astype(np.float16)
    # condition tiled: ct[p, j] = cond[0, j*128 + p]
    ct = np.ascontiguousarray(
        condition.reshape(JT, 128).T).astype(np.float16)
    # mask: fold 4 col-tile partial rows back to batch rows; carries 1/WSCALE
    mask = np.zeros((128, B), np.float16)
    for p in range(128):
        mask[p, p % 32] = 1.0 / WSCALE
    b2r = np.ascontiguousarray(b2.reshape(1, K)).astype(f)

    in_maps = []
    for c in range(NCORES):
        osl = slice(c * OC, (c + 1) * OC)
        hsl = slice(c * HS, (c + 1) * HS)
        # W shard [k, o, i] -> [k, p, j*OC + o] with i = j*128 + p
        wt = np.ascontiguousarray(
            kernels_weights[:, osl, :].reshape(K, OC, JT, 128)
            .transpose(0, 3, 2, 1)).reshape(K, 128, JT * OC)
        wt = (wt * WSCALE).astype(fp8)
        kbs = (np.ascontiguousarray(kernels_bias[:, osl]).reshape(1, K * OC)
               * WSCALE).astype(fp8)
        # w1 shard tiled: w1st[p, j*HS + h] = w1[j*128 + p, c*HS + h]
        w1st = np.ascontiguousarray(
            w1[:, hsl].reshape(JT, 128, HS).transpose(1, 0, 2)
        ).reshape(128, JT * HS)
        w1st = (w1st * W1SCALE).astype(fp8)
        # w2 shard, zero-padded to 128 rows; carries 1/W1SCALE
        w2st = np.zeros((128, K), np.float32)
        w2st[:HS] = w2[hsl] / W1SCALE
        w2st = w2st.astype(bf16)
        b1c = np.ascontiguousarray(
            b1[hsl].reshape(HS, 1) * W1SCALE).astype(f)
        in_maps.append({
            "wt": wt, "kbs": kbs, "xt": xt, "ct": ct,
            "w1st": w1st, "w2st": w2st, "b1c": b1c, "b2r": b2r,
            "mask": mask,
        })
    return in_maps


def kernel(x, condition, w1, b1, w2, b2, kernels_weights, kernels_bias):
    global LAST_RESULTS
    if "nc" not in _CACHE:
        _CACHE["nc"] = _build_module()
    nc = _CACHE["nc"]

    in_maps = _prep_inputs(x, condition, w1, b1, w2, b2,
                           kernels_weights, kernels_bias)

    res = run_bass_kernel_spmd(nc, in_maps, core_ids=list(range(NCORES)))
    LAST_RESULTS = res

    out = np.concatenate([res.results[c]["y"] for c in range(NCORES)], axis=1)
    return np.ascontiguousarray(out, dtype=np.float32)


if __name__ == "__main__":
    rng = np.random.default_rng(0)
    ins = {
        "x": rng.standard_normal((B, IN), dtype=np.float32),
        "condition": rng.standard_normal((1, IN), dtype=np.float32),
        "w1": rng.standard_normal((IN, H), dtype=np.float32) * 0.02,
        "b1": np.zeros(H, np.float32),
        "w2": rng.standard_normal((H, K), dtype=np.float32) * 0.02,
        "b2": np.zeros(K, np.float32),
        "kernels_weights": rng.standard_normal((K, OUT, IN),
                                               dtype=np.float32) * 0.01,
        "kernels_bias": np.zeros((K, OUT), np.float32),
    }
    y = kernel(**ins)
    print("out", y.shape, y.dtype, float(np.abs(y).mean()))


# revision 42
# speedup vs baseline: 2.1367x; 2.1367x over previous
"""Trainium2 Bass kernel for DynamicCondLinear (MoE-routing style).

Math: condition batch is 1, so the softmax routing weights (K=8) are shared by
all 32 samples; out = sum_k a_k * (x @ W_k^T + kb_k) with
a = softmax(relu(cond @ w1 + b1) @ w2 + b2).

v4 design (vs the 44 us fp16 baseline):
 - Weight stream in fp8 E3M4 (4-bit mantissa): half the HBM bytes of fp16,
   measured end-to-end rel err ~1.15e-2 (gate is 2e-2). W is pre-scaled by
   2^8 on the host (exact pow2), folded back via the mask constant.
 - UNSCALED xT stationaries: the main matmuls do not depend on the softmax.
   Per-expert psum regions accumulate y_k = x @ W_k^T; the alphas are
   applied at PSUM-evac time with fused scale-add ops on DVE, so the alpha
   chain only gates the last ~1 us of the kernel.
 - Col-tiled main matmuls: out rows for j%4==t live at psum partitions
   32t..32t+31, so 4 independent 128x32 PE tiles stream concurrently
   (PE busy ~4 us << DMA ~15 us). A final mask-matmul folds the 4
   partition groups back to (32, 256).
 - The alpha MLP keeps w1 replicated but in fp8 E3M4 (1 MB/core); scores
   are computed without any PE-mode switches: h row via 128-row matmuls,
   then s_k = sum(h * w2_k) on DVE using tensor_tensor's accum_out.
   (A cross-core sharded-MLP variant was measured: both collective_compute
   (~53 us) and SWDGE remote DMA (~40 us + unaligned core starts) are too
   slow on this stack, so replication wins.)
 - kernels_bias enters the main accumulation as an extra contract row
   (xe row-0-ones stationary vs a kb slab); b1 likewise via xe against a
   row0-only w1b slab. No transposes anywhere.

Host-side prep is layout/cast only (transpose/reshape/pow2-scale/cast).
"""

import os
import sys

import numpy as np

if "/opt/trn_rl_repo" not in sys.path:
    sys.path.insert(0, "/opt/trn_rl_repo")

import concourse.bacc as bacc
import concourse.mybir as mybir
import concourse.tile as tile
from concourse.bass_utils import run_bass_kernel_spmd

B, IN, OUT, K, H = 32, 2048, 2048, 8, 512
NCORES = 8
OC = OUT // NCORES   # 256 out channels per core
JT = IN // 128       # 16 contraction tiles

F32 = mybir.dt.float32
BF16 = mybir.dt.bfloat16
FP16 = mybir.dt.float16
FP8 = mybir.dt.float8e3  # E3M4

WSCALE = 2.0 ** 8    # W * 256 -> e3m4 sweet spot; folded into mask constant
W1SCALE = 2.0 ** 5   # w1 * 32; folded into w2 host-side

COLTILE = os.environ.get("KERNEL_COLTILE", "1") == "1"
NWARM = int(os.environ.get("KERNEL_WARMUP", "16"))

_CACHE = {}
LAST_RESULTS = None  # test.py reads this for profiling info


def _build_module():
    nc = bacc.Bacc("TRN2", target_bir_lowering=False, debug=False,
                   num_devices=NCORES)

    wt_d = nc.dram_tensor("wt", (K, 128, JT * OC), FP8, kind="ExternalInput")
    kbs_d = nc.dram_tensor("kbs", (1, K * OC), FP8, kind="ExternalInput")
    xt_d = nc.dram_tensor("xt", (128, JT * B), FP16, kind="ExternalInput")
    ct_d = nc.dram_tensor("ct", (128, JT), FP16, kind="ExternalInput")
    w1t_d = nc.dram_tensor("w1t", (128, JT * H), FP8, kind="ExternalInput")
    w1b_d = nc.dram_tensor("w1b", (1, H), FP8, kind="ExternalInput")
    w2r_d = nc.dram_tensor("w2r", (1, K * H), BF16, kind="ExternalInput")
    b2r_d = nc.dram_tensor("b2r", (1, K), F32, kind="ExternalInput")
    mask_d = nc.dram_tensor("mask", (128, B), FP16, kind="ExternalInput")
    y_d = nc.dram_tensor("y", (B, OC), F32, kind="ExternalOutput")
    # warmup sink: consumed so bacc's DCE keeps the PE warm-up matmuls
    ysink_d = nc.dram_tensor("ysink", (1, 1), F32, kind="ExternalOutput")
    DEBUG_DUMP = os.environ.get("KERNEL_DEBUG", "0") == "1"
    if DEBUG_DUMP:
        dbg_d = nc.dram_tensor("dbg", (4, H), F32, kind="ExternalOutput")
        dbg2_d = nc.dram_tensor("dbg2", (3, K * OC), F32,
                                kind="ExternalOutput")

    # Raw PSUM for the 8 expert accumulation regions: 4 banks, guaranteed
    # disjoint from the pool-managed psum tiles (a pool allocation overlap
    # was observed to clobber region 0).
    yk_psum = nc.alloc_psum_tensor("yk_psum", [128, K * OC], F32)

    with tile.TileContext(nc) as tc:
        with (
            tc.tile_pool(name="cpool", bufs=1) as cpool,
            tc.tile_pool(name="wpool", bufs=1) as wpool,
            tc.tile_pool(name="ppool", bufs=1, space="PSUM") as ppool,
        ):
            # ---------- small loads on the scalar (ACT) ring ----------
            xt_sb = cpool.tile((128, JT * B), FP16)
            nc.scalar.dma_start(xt_sb[:], xt_d.ap())
            ct_sb = cpool.tile((128, JT), FP16)
            nc.scalar.dma_start(ct_sb[:], ct_d.ap())
            w2r_sb = cpool.tile((1, K * H), BF16)
            nc.scalar.dma_start(w2r_sb[:], w2r_d.ap())
            b2r_sb = cpool.tile((1, K), F32)
            nc.scalar.dma_start(b2r_sb[:], b2r_d.ap())
            mask_sb = cpool.tile((128, B), FP16)
            nc.scalar.dma_start(mask_sb[:], mask_d.ap())
            w1b_sb = cpool.tile((128, H), FP8)
            nc.vector.memset(w1b_sb[:], 0.0)
            nc.scalar.dma_start(w1b_sb[0:1, :], w1b_d.ap())

            # ---------- weight stream on the sync ring ----------
            # order: W0, W1, w1t (alpha MLP), W2..W7, kbs (bias rows)
            slabs = [None] * K
            w1t_sb = None
            kbs_sb = None
            for k in range(K):
                slab = wpool.tile((128, JT * OC), FP8, tag="wt_slab", bufs=K)
                nchunk = 4 if k == 0 else 2
                q = JT * OC // nchunk
                for c in range(nchunk):
                    nc.sync.dma_start(slab[:, c * q:(c + 1) * q],
                                      wt_d.ap()[k][:, c * q:(c + 1) * q])
                slabs[k] = slab
                if k == 0:
                    kbs_sb = wpool.tile((128, K * OC), FP8, tag="kbs")
                    nc.vector.memset(kbs_sb[:], 0.0)
                    nc.sync.dma_start(kbs_sb[0:1, :], kbs_d.ap())
                if k == 1:
                    w1t_sb = wpool.tile((128, JT * H), FP8, tag="w1t")
                    hj = JT * H // 2
                    nc.sync.dma_start(w1t_sb[:, :hj], w1t_d.ap()[:, :hj])
                    nc.sync.dma_start(w1t_sb[:, hj:], w1t_d.ap()[:, hj:])

            # xe stationary: row 0 = ones (bias contract row)
            xe_sb = cpool.tile((128, B), FP16)
            nc.gpsimd.memset(xe_sb[:], 0.0)
            nc.gpsimd.memset(xe_sb[0:1, :], 1.0)

            # ---------- PE warm-up (HAM): dep-free matmuls ----------
            dum_a = cpool.tile((128, B), FP16)
            nc.gpsimd.memset(dum_a[:], 0.0)
            dum_b = cpool.tile((128, OC), FP8)
            nc.gpsimd.memset(dum_b[:], 0.0)
            dum_psum = ppool.tile((B, OC), F32, tag="pdum")
            dum_sink = cpool.tile((1, 1), F32)
            for _ in range(NWARM):
                nc.tensor.matmul(dum_psum[:], dum_a[:], dum_b[:],
                                 start=True, stop=True)

            # ---------- main stream + alpha MLP, interleaved in PE order ---
            # YK[32t+b, k*OC+o] accumulates sum_{j: j%4==t} xT_j.T @ W_k[j].
            # The per-expert bias mm (xe row vs kbs) lands in tile 0's rows
            # and closes tile 0's group; issued a few experts late so the
            # kbs DMA (last in the stream) never stalls the PE queue.
            psum_h = ppool.tile((1, H), F32, tag="ph")

            def expert_mms(k):
                kc = slice(k * OC, (k + 1) * OC)
                for j in range(JT):
                    t = j % 4 if COLTILE else 0
                    rows = slice(32 * t, 32 * t + 32)
                    if COLTILE:
                        first = j < 4
                        last = (j >= JT - 4) and t != 0
                    else:
                        first = j == 0
                        last = False
                    nc.tensor.matmul(
                        yk_psum[rows, kc],
                        xt_sb[:, j * B:(j + 1) * B],
                        slabs[k][:, j * OC:(j + 1) * OC],
                        start=first, stop=last,
                        tile_position=(0, 32 * t) if COLTILE else None,
                    )

            def bias_mm(k):
                kc = slice(k * OC, (k + 1) * OC)
                nc.tensor.matmul(yk_psum[0:32, kc], xe_sb[:], kbs_sb[:, kc],
                                 start=False, stop=True)

            expert_mms(0)
            bias_mm(0)
            expert_mms(1)
            bias_mm(1)

            # alpha MLP: h row = relu(cond @ w1 + b1), all 128-row matmuls
            for j in range(JT):
                nc.tensor.matmul(
                    psum_h[:],
                    ct_sb[:, j:j + 1],
                    w1t_sb[:, j * H:(j + 1) * H],
                    start=(j == 0), stop=False,
                )
            nc.tensor.matmul(psum_h[:], xe_sb[:, 0:1], w1b_sb[:],
                             start=False, stop=True)
            h_sb = cpool.tile((1, H), FP16)
            nc.scalar.activation(h_sb[:], psum_h[:],
                                 mybir.ActivationFunctionType.Relu)

            # scores on DVE: s_k = sum(h * w2_k) via accum_out (no PE)
            s_sb = cpool.tile((1, K), F32)
            hw2 = cpool.tile((1, H), F32)
            for k in range(K):
                nc.vector.affine_mul_reduce(hw2[:], s_sb[:, k:k + 1],
                                            h_sb[:],
                                            w2r_sb[:, k * H:(k + 1) * H],
                                            1.0, 0.0)
            sb2_sb = cpool.tile((1, K), F32)
            nc.vector.tensor_add(sb2_sb[:], s_sb[:, 0:K], b2r_sb[:])
            s_b = cpool.tile((128, K), F32)
            nc.gpsimd.partition_broadcast(s_b[:], sb2_sb[:])
            e_b = cpool.tile((128, K), F32)
            nc.scalar.activation(e_b[:], s_b[:],
                                 mybir.ActivationFunctionType.Exp)
            esum = cpool.tile((128, 1), F32)
            nc.vector.reduce_sum(esum[:], e_b[:], axis=mybir.AxisListType.X)
            rinv = cpool.tile((128, 1), F32)
            nc.vector.reciprocal(rinv[:], esum[:])

            # rest of the expert stream; bias mms trail by 3 experts
            comb = []  # deferred combine closures

            PR = 128 if COLTILE else 32
            s_acc0 = cpool.tile((128, OC), FP16)
            s_acc1 = cpool.tile((128, OC), FP16)
            s_accs = [s_acc0, s_acc1]
            state = {"prev": None}

            def combine(k):
                kc = slice(k * OC, (k + 1) * OC)
                dst = s_accs[k % 2]
                prev = state["prev"]
                if prev is None:
                    nc.vector.tensor_scalar_mul(
                        dst[0:PR, :], yk_psum[0:PR, kc], e_b[0:PR, 0:1])
                else:
                    nc.vector.scalar_tensor_tensor(
                        dst[0:PR, :], yk_psum[0:PR, kc], e_b[0:PR, k:k + 1],
                        prev[0:PR, :],
                        op0=mybir.AluOpType.mult, op1=mybir.AluOpType.add)
                state["prev"] = dst

            # each expert's groups are fully closed (bias mm inline) before
            # the next expert's start=True mms touch the shared psum bank;
            # the DVE combine trails two experts behind for the same reason.
            for k in range(2, K):
                expert_mms(k)
                bias_mm(k)
                combine(k - 2)
            combine(K - 2)
            combine(K - 1)
            prev = state["prev"]

            y_sb = cpool.tile((B, OC), F32)
            if COLTILE:
                # fold the 4 col-tile partition groups: y = mask.T @ S
                # (mask carries the 1/WSCALE factor)
                yt_psum = ppool.tile((B, OC), F32, tag="pyt")
                nc.tensor.matmul(yt_psum[:], mask_sb[:], prev[:],
                                 start=True, stop=True)
                nc.scalar.activation(y_sb[:], yt_psum[:],
                                     mybir.ActivationFunctionType.Copy,
                                     scale=rinv[0:B, :])
            else:
                rinv2 = cpool.tile((128, 1), F32)
                nc.vector.tensor_scalar_mul(rinv2[:], rinv[:], 1.0 / WSCALE)
                nc.scalar.activation(y_sb[:], prev[0:B, :],
                                     mybir.ActivationFunctionType.Copy,
                                     scale=rinv2[0:B, :])
            nc.scalar.dma_start(y_d.ap(), y_sb[:])
            nc.vector.tensor_copy(dum_sink[:], dum_psum[0:1, 0:1])
            nc.scalar.dma_start(ysink_d.ap(), dum_sink[:])
            if DEBUG_DUMP:
                hf = cpool.tile((1, H), F32)
                nc.vector.tensor_copy(hf[:], h_sb[:])
                nc.scalar.dma_start(dbg_d.ap()[0:1, :], hf[:])
                nc.scalar.dma_start(dbg_d.ap()[1:2, 0:K], s_sb[:])
                nc.scalar.dma_start(dbg_d.ap()[2:3, 0:K], e_b[0:1, :])
                nc.scalar.dma_start(dbg_d.ap()[3:4, 0:1], esum[0:1, :])
                ykrow = cpool.tile((1, K * OC), F32)
                nc.vector.tensor_copy(ykrow[:], yk_psum[0:1, :])
                nc.scalar.dma_start(dbg2_d.ap()[0:1, :], ykrow[:])

                saccrow = cpool.tile((1, OC), F32)
                nc.vector.tensor_copy(saccrow[:], prev[0:1, :])
                nc.scalar.dma_start(dbg2_d.ap()[2:3, 0:OC], saccrow[:])

    nc.compile()
    return nc


def _prep_inputs(x, condition, w1, b1, w2, b2, kernels_weights, kernels_bias):
    """Layout-only host prep: per-core shards, retile/cast for DMA."""
    import ml_dtypes
    bf16 = ml_dtypes.bfloat16
    fp8 = ml_dtypes.float8_e3m4
    f = np.float32
    x = np.asarray(x, f)
    condition = np.asarray(condition, f)
    w1 = np.asarray(w1, f)
    b1 = np.asarray(b1, f)
    w2 = np.asarray(w2, f)
    b2 = np.asarray(b2, f)
    kernels_weights = np.asarray(kernels_weights, f)
    kernels_bias = np.asarray(kernels_bias, f)

    # xT tiled: xt[p, j*B + b] = x[b, j*128 + p]
    xt = np.ascontiguousarray(
        x.T.reshape(JT, 128, B).transpose(1, 0, 2)).reshape(128, JT * B)
    xt = xt.astype(np.float16)
    # condition tiled: ct[p, j] = cond[0, j*128 + p]
    ct = np.ascontiguousarray(
        condition.reshape(JT, 128).T).astype(np.float16)
    # w1 tiled: w1t[p, j*H + h] = w1[j*128 + p, h] * 2^5  (fp8 e3m4)
    w1t = np.ascontiguousarray(
        w1.reshape(JT, 128, H).transpose(1, 0, 2)).reshape(128, JT * H)
    w1t = (w1t * W1SCALE).astype(fp8)
    w1b = (np.ascontiguousarray(b1.reshape(1, H)) * W1SCALE).astype(fp8)
    # w2 rows per expert, carrying 1/W1SCALE: w2r[0, k*H + q] = w2[q, k]/32
    w2r = (np.ascontiguousarray(w2.T.reshape(1, K * H)) / W1SCALE).astype(bf16)
    b2r = np.ascontiguousarray(b2.reshape(1, K)).astype(f)
    # mask: fold 4 col-tile partial rows back to batch rows; carries 1/WSCALE
    mask = np.zeros((128, B), np.float16)
    for p in range(128):
        mask[p, p % 32] = 1.0 / WSCALE

    in_maps = []
    for c in range(NCORES):
        osl = slice(c * OC, (c + 1) * OC)
        # W shard [k, o, i] -> [k, p, j*OC + o] with i = j*128 + p
        wt = np.ascontiguousarray(
            kernels_weights[:, osl, :].reshape(K, OC, JT, 128)
            .transpose(0, 3, 2, 1)).reshape(K, 128, JT * OC)
        wt = (wt * WSCALE).astype(fp8)
        kbs = (np.ascontiguousarray(kernels_bias[:, osl]).reshape(1, K * OC)
               * WSCALE).astype(fp8)
        in_maps.append({
            "wt": wt, "kbs": kbs, "xt": xt, "ct": ct,
            "w1t": w1t, "w1b": w1b, "w2r": w2r, "b2r": b2r,
            "mask": mask,
        })
    return in_maps


def kernel(x, condition, w1, b1, w2, b2, kernels_weights, kernels_bias):
    global LAST_RESULTS
    if "nc" not in _CACHE:
        _CACHE["nc"] = _build_module()
    nc = _CACHE["nc"]

    in_maps = _prep_inputs(x, condition, w1, b1, w2, b2,
                           kernels_weights, kernels_bias)

    res = run_bass_kernel_spmd(nc, in_maps, core_ids=list(range(NCORES)))
    LAST_RESULTS = res

    out = np.concatenate([res.results[c]["y"] for c in range(NCORES)], axis=1)
    return np.ascontiguousarray(out, dtype=np.float32)


if __name__ == "__main__":
    rng = np.random.default_rng(0)
    ins = {
        "x": rng.standard_normal((B, IN), dtype=np.float32),
        "condition": rng.standard_normal((1, IN), dtype=np.float32),
        "w1": rng.standard_normal((IN, H), dtype=np.float32) * 0.02,
        "b1": np.zeros(H, np.float32),
        "w2": rng.standard_normal((H, K), dtype=np.float32) * 0.02,
        "b2": np.zeros(K, np.float32),
        "kernels_weights": rng.standard_normal((K, OUT, IN),
                                               dtype=np.float32) * 0.01,
        "kernels_bias": np.zeros((K, OUT), np.float32),
    }
    y = kernel(**ins)
    print("out", y.shape, y.dtype, float(np.abs(y).mean()))
